# revision 6
# baseline (speedup 1.0000x reference)
"""CrossLayerTranscoder Trainium2 kernel.

Math (reference):
  feats = relu(einsum('lbnh,lfh->lbnf', resid, enc_w))            [L,B,S,F]
  recon[t] = sum_{s<=t} feats[s] @ dec_w[s,t].T                   [L,B,S,H]

Sharding: token dim (B*S = 2048) is fully data-parallel through both
stages (recon couples layers/features, never tokens), so each of the 8
cores handles 256 tokens end-to-end with zero communication and zero
redundant compute.  Each core streams the 136 causal (s,t) dec_w pairs
(t-major) from its own HBM copy.

Compute is bf16 (4x faster than fp32 on the PE; accumulation stays f32
in PSUM).  Host pre-transposes operands into PE-friendly layouts:
  residT  [L, 128p, 16a, tok]   h = a*128+p   (encode rhs, moving)
  enc_wT  [L, 128p, 16a, F]     h = a*128+p   (encode lhsT, stationary)
  dec_wT  [136, 4k, 128p, H]    f = k*128+p   (decode rhs, moving)
Encode produces feats in [f, tok] layout so decode can use feats tiles
directly as the stationary operand: recon[tok, h] += feats[f, tok].T @
dec_w[f, h].
"""

import sys

sys.path.insert(0, "/opt/trn_rl_repo")

import os
import numpy as np
import ml_dtypes

L, B, S, H, F = 16, 4, 512, 2048, 512
NCORES = 8
TOK = B * S               # 2048 tokens
TOKC = TOK // NCORES      # 256 tokens per core
HT = H // 128             # 16 h-tiles (encode contraction)
FT = F // 128             # 4 f-tiles (decode contraction)
NPAIRS = L * (L + 1) // 2 # 136 causal (s,t) pairs
HB = H // 512             # 4 h-blocks of 512 (decode moving free dim)
NTT = TOKC // 128         # 2 token tiles of 128 (decode output partitions)

BF16 = ml_dtypes.bfloat16

# pair index p(t,s) = t(t+1)/2 + s, ordered t-major then s ascending
PAIRS = [(s, t) for t in range(L) for s in range(t + 1)]


def _build_nc():
    import concourse.bass as bass
    import concourse.bacc as bacc
    import concourse.mybir as mybir
    from concourse import tile

    f32 = mybir.dt.float32
    bf16 = mybir.dt.bfloat16

    nc = bacc.Bacc()
    residT = nc.declare_dram_parameter("residT", [L, 128, HT, TOKC], bf16, isOutput=False)
    enc_wT = nc.declare_dram_parameter("enc_wT", [L, 128, HT, F], bf16, isOutput=False)
    dec_wT = nc.declare_dram_parameter("dec_wT", [NPAIRS, FT, 128, H], bf16, isOutput=False)
    feats_o = nc.declare_dram_parameter("feats", [L, F, TOKC], bf16, isOutput=True)
    recon_o = nc.declare_dram_parameter("recon", [L, TOKC, H], bf16, isOutput=True)

    with tile.TileContext(nc) as tc:
        with (
            tc.tile_pool(name="rp", bufs=3) as rp,
            tc.tile_pool(name="ep", bufs=2) as ep,
            tc.tile_pool(name="fp", bufs=L * FT) as fp,
            tc.tile_pool(name="dp", bufs=16) as dp,
            tc.tile_pool(name="op", bufs=3) as op,
            tc.tile_pool(name="ps", bufs=8, space=bass.MemorySpace.PSUM) as ps,
        ):
            feats_tiles = {}

            def encode(s):
                rt = rp.tile([128, HT, TOKC], bf16, tag="rt", name=f"rt{s}")
                nc.sync.dma_start(rt[:, 0:HT // 2, :], residT[s, :, 0:HT // 2, :])
                nc.sync.dma_start(rt[:, HT // 2:, :], residT[s, :, HT // 2:, :])
                et = ep.tile([128, HT, F], bf16, tag="et", name=f"et{s}")
                nc.sync.dma_start(et[:, 0:HT // 2, :], enc_wT[s, :, 0:HT // 2, :])
                nc.sync.dma_start(et[:, HT // 2:, :], enc_wT[s, :, HT // 2:, :])
                for f in range(FT):
                    acc = ps.tile([128, TOKC], f32, tag="ps", name=f"acc{s}_{f}")
                    for k in range(HT):
                        nc.tensor.matmul(
                            acc[:],
                            et[:, k, f * 128:(f + 1) * 128],
                            rt[:, k, :],
                            start=(k == 0),
                            stop=(k == HT - 1),
                        )
                    ft = fp.tile([128, TOKC], bf16, tag="ft", name=f"ft{s}_{f}")
                    nc.scalar.activation(ft[:], acc[:], mybir.ActivationFunctionType.Relu)
                    nc.sync.dma_start(feats_o[s, f * 128:(f + 1) * 128, :], ft[:])
                    feats_tiles[(s, f)] = ft

            def wave(t):
                banks = [
                    ps.tile([128, 512], f32, tag="ps", name=f"bank{t}_{i}")
                    for i in range(NTT * HB)
                ]
                for s in range(t + 1):
                    pair = t * (t + 1) // 2 + s
                    for k in range(FT):
                        dw = dp.tile([128, H], bf16, tag="dw", name=f"dw{t}_{s}_{k}")
                        nc.sync.dma_start(dw[:], dec_wT[pair, k])
                        first = (s == 0 and k == 0)
                        last = (s == t and k == FT - 1)
                        for tt in range(NTT):
                            lhsT = feats_tiles[(s, k)][:, tt * 128:(tt + 1) * 128]
                            for hb in range(HB):
                                nc.tensor.matmul(
                                    banks[tt * HB + hb][:],
                                    lhsT,
                                    dw[:, hb * 512:(hb + 1) * 512],
                                    start=first,
                                    stop=last,
                                )
                for tt in range(NTT):
                    stg = op.tile([128, H], bf16, tag="stg", name=f"stg{t}_{tt}")
                    for hb in range(HB):
                        nc.vector.tensor_copy(
                            stg[:, hb * 512:(hb + 1) * 512], banks[tt * HB + hb][:]
                        )
                    nc.sync.dma_start(recon_o[t, tt * 128:(tt + 1) * 128, :], stg[:])

            # stagger: encode(t+1) sits between wave(t-1) and wave(t), so
            # PSUM banks of the previous wave drain while encode runs, and
            # decode matmuls keep PE busy while encode inputs stream in.
            encode(0)
            encode(1)
            for t in range(L):
                wave(t)
                if t + 2 < L:
                    encode(t + 2)

    nc.finalize()
    return nc


_NC_CACHE = None


def _get_nc():
    global _NC_CACHE
    if _NC_CACHE is None:
        _NC_CACHE = _build_nc()
    return _NC_CACHE


def _prep_inputs(resid, enc_w, dec_w):
    # residT per core: [L, 128, HT, TOKC], h = a*128 + p
    rs = resid.reshape(L, TOK, HT, 128).transpose(0, 3, 2, 1).astype(BF16)
    resid_shards = [np.ascontiguousarray(rs[..., c * TOKC:(c + 1) * TOKC]) for c in range(NCORES)]
    # enc_wT: [L, 128, HT, F]
    ew = np.ascontiguousarray(enc_w.reshape(L, F, HT, 128).transpose(0, 3, 2, 1)).astype(BF16)
    # dec_wT: [NPAIRS, FT, 128, H], f = k*128 + p
    dw = np.empty((NPAIRS, FT, 128, H), dtype=BF16)
    for i, (s, t) in enumerate(PAIRS):
        dw[i] = dec_w[s, t].T.astype(BF16).reshape(FT, 128, H)
    return resid_shards, ew, dw


def kernel(resid, enc_w, dec_w):
    from concourse.bass_utils import run_bass_kernel_spmd

    resid_shards, ew, dw = _prep_inputs(resid, enc_w, dec_w)
    nc = _get_nc()
    in_maps = [
        {"residT": resid_shards[c], "enc_wT": ew, "dec_wT": dw} for c in range(NCORES)
    ]
    trace = os.environ.get("TRN_KERNEL_PROFILE", "") == "1"
    res = run_bass_kernel_spmd(nc, in_maps, list(range(NCORES)), trace=trace)
    if trace:
        print(f"HW exec time: {res.exec_time_ns} ns")
        kernel.last_exec_time_ns = res.exec_time_ns
        kernel.last_results = res

    feats = np.concatenate([r["feats"] for r in res.results], axis=2)  # [L, F, TOK]
    feats = feats.astype(np.float32).transpose(0, 2, 1).reshape(L, B, S, F)
    recon = np.concatenate([r["recon"] for r in res.results], axis=1)  # [L, TOK, H]
    recon = recon.astype(np.float32).reshape(L, B, S, H)
    return feats, recon


# revision 7
# speedup vs baseline: 1.0418x; 1.0418x over previous
"""CrossLayerTranscoder Trainium2 kernel.

Math (reference):
  feats = relu(einsum('lbnh,lfh->lbnf', resid, enc_w))            [L,B,S,F]
  recon[t] = sum_{s<=t} feats[s] @ dec_w[s,t].T                   [L,B,S,H]

Sharding: token dim (B*S = 2048) is fully data-parallel through both
stages (recon couples layers/features, never tokens), so each of the 8
cores handles 256 tokens end-to-end with zero communication and zero
redundant compute.  Each core streams the 136 causal (s,t) dec_w pairs
(t-major) from its own HBM copy.

Compute is bf16 (4x faster than fp32 on the PE; accumulation stays f32
in PSUM).  Host pre-transposes operands into PE-friendly layouts:
  residT  [L, 128p, 16a, tok]   h = a*128+p   (encode rhs, moving)
  enc_wT  [L, 128p, 16a, F]     h = a*128+p   (encode lhsT, stationary)
  dec_wT  [136, 4k, 128p, H]    f = k*128+p   (decode rhs, moving)
Encode produces feats in [f, tok] layout so decode can use feats tiles
directly as the stationary operand: recon[tok, h] += feats[f, tok].T @
dec_w[f, h].
"""

import sys

sys.path.insert(0, "/opt/trn_rl_repo")

import os
import numpy as np
import ml_dtypes

L, B, S, H, F = 16, 4, 512, 2048, 512
NCORES = 8
TOK = B * S               # 2048 tokens
TOKC = TOK // NCORES      # 256 tokens per core
HT = H // 128             # 16 h-tiles (encode contraction)
FT = F // 128             # 4 f-tiles (decode contraction)
NPAIRS = L * (L + 1) // 2 # 136 causal (s,t) pairs
HB = H // 512             # 4 h-blocks of 512 (decode moving free dim)
NTT = TOKC // 128         # 2 token tiles of 128 (decode output partitions)

BF16 = ml_dtypes.bfloat16

# pair index p(t,s) = t(t+1)/2 + s, ordered t-major then s ascending
PAIRS = [(s, t) for t in range(L) for s in range(t + 1)]


def _build_nc():
    import concourse.bass as bass
    import concourse.bacc as bacc
    import concourse.mybir as mybir
    from concourse import tile

    f32 = mybir.dt.float32
    bf16 = mybir.dt.bfloat16

    nc = bacc.Bacc()
    residT = nc.declare_dram_parameter("residT", [L, 128, HT, TOKC], bf16, isOutput=False)
    enc_wT = nc.declare_dram_parameter("enc_wT", [L, 128, HT, F], bf16, isOutput=False)
    dec_wT = nc.declare_dram_parameter("dec_wT", [NPAIRS, FT, 128, H], bf16, isOutput=False)
    feats_o = nc.declare_dram_parameter("feats", [L, F, TOKC], bf16, isOutput=True)
    recon_o = nc.declare_dram_parameter("recon", [L, TOKC, H], bf16, isOutput=True)

    with tile.TileContext(nc) as tc:
        with (
            tc.tile_pool(name="rp", bufs=3) as rp,
            tc.tile_pool(name="ep", bufs=2) as ep,
            tc.tile_pool(name="fp", bufs=L * FT) as fp,
            tc.tile_pool(name="dp", bufs=16) as dp,
            tc.tile_pool(name="op", bufs=3) as op,
            tc.tile_pool(name="ps", bufs=8, space=bass.MemorySpace.PSUM) as ps,
        ):
            feats_tiles = {}

            def encode(s):
                rt = rp.tile([128, HT, TOKC], bf16, tag="rt", name=f"rt{s}")
                nc.scalar.dma_start(rt[:, 0:HT // 2, :], residT[s, :, 0:HT // 2, :])
                nc.scalar.dma_start(rt[:, HT // 2:, :], residT[s, :, HT // 2:, :])
                et = ep.tile([128, HT, F], bf16, tag="et", name=f"et{s}")
                nc.scalar.dma_start(et[:, 0:HT // 2, :], enc_wT[s, :, 0:HT // 2, :])
                nc.scalar.dma_start(et[:, HT // 2:, :], enc_wT[s, :, HT // 2:, :])
                for f in range(FT):
                    acc = ps.tile([128, TOKC], f32, tag="ps", name=f"acc{s}_{f}")
                    for k in range(HT):
                        nc.tensor.matmul(
                            acc[:],
                            et[:, k, f * 128:(f + 1) * 128],
                            rt[:, k, :],
                            start=(k == 0),
                            stop=(k == HT - 1),
                        )
                    ft = fp.tile([128, TOKC], bf16, tag="ft", name=f"ft{s}_{f}")
                    nc.scalar.activation(ft[:], acc[:], mybir.ActivationFunctionType.Relu)
                    nc.scalar.dma_start(feats_o[s, f * 128:(f + 1) * 128, :], ft[:])
                    feats_tiles[(s, f)] = ft

            def wave(t):
                banks = [
                    ps.tile([128, 512], f32, tag="ps", name=f"bank{t}_{i}")
                    for i in range(NTT * HB)
                ]
                for s in range(t + 1):
                    pair = t * (t + 1) // 2 + s
                    for k in range(FT):
                        dw = dp.tile([128, H], bf16, tag="dw", name=f"dw{t}_{s}_{k}")
                        nc.sync.dma_start(dw[:], dec_wT[pair, k])
                        first = (s == 0 and k == 0)
                        last = (s == t and k == FT - 1)
                        for tt in range(NTT):
                            lhsT = feats_tiles[(s, k)][:, tt * 128:(tt + 1) * 128]
                            for hb in range(HB):
                                nc.tensor.matmul(
                                    banks[tt * HB + hb][:],
                                    lhsT,
                                    dw[:, hb * 512:(hb + 1) * 512],
                                    start=first,
                                    stop=last,
                                )
                for tt in range(NTT):
                    stg = op.tile([128, H], bf16, tag="stg", name=f"stg{t}_{tt}")
                    for hb in range(HB):
                        nc.vector.tensor_copy(
                            stg[:, hb * 512:(hb + 1) * 512], banks[tt * HB + hb][:]
                        )
                    nc.scalar.dma_start(recon_o[t, tt * 128:(tt + 1) * 128, :], stg[:])

            # stagger: encode(t+1) sits between wave(t-1) and wave(t), so
            # PSUM banks of the previous wave drain while encode runs, and
            # decode matmuls keep PE busy while encode inputs stream in.
            encode(0)
            encode(1)
            for t in range(L):
                wave(t)
                if t + 2 < L:
                    encode(t + 2)

    nc.finalize()
    return nc


_NC_CACHE = None


def _get_nc():
    global _NC_CACHE
    if _NC_CACHE is None:
        _NC_CACHE = _build_nc()
    return _NC_CACHE


def _prep_inputs(resid, enc_w, dec_w):
    # residT per core: [L, 128, HT, TOKC], h = a*128 + p
    rs = resid.reshape(L, TOK, HT, 128).transpose(0, 3, 2, 1).astype(BF16)
    resid_shards = [np.ascontiguousarray(rs[..., c * TOKC:(c + 1) * TOKC]) for c in range(NCORES)]
    # enc_wT: [L, 128, HT, F]
    ew = np.ascontiguousarray(enc_w.reshape(L, F, HT, 128).transpose(0, 3, 2, 1)).astype(BF16)
    # dec_wT: [NPAIRS, FT, 128, H], f = k*128 + p
    dw = np.empty((NPAIRS, FT, 128, H), dtype=BF16)
    for i, (s, t) in enumerate(PAIRS):
        dw[i] = dec_w[s, t].T.astype(BF16).reshape(FT, 128, H)
    return resid_shards, ew, dw


def kernel(resid, enc_w, dec_w):
    from concourse.bass_utils import run_bass_kernel_spmd

    resid_shards, ew, dw = _prep_inputs(resid, enc_w, dec_w)
    nc = _get_nc()
    in_maps = [
        {"residT": resid_shards[c], "enc_wT": ew, "dec_wT": dw} for c in range(NCORES)
    ]
    trace = os.environ.get("TRN_KERNEL_PROFILE", "") == "1"
    res = run_bass_kernel_spmd(nc, in_maps, list(range(NCORES)), trace=trace)
    if trace:
        print(f"HW exec time: {res.exec_time_ns} ns")
        kernel.last_exec_time_ns = res.exec_time_ns
        kernel.last_results = res

    feats = np.concatenate([r["feats"] for r in res.results], axis=2)  # [L, F, TOK]
    feats = feats.astype(np.float32).transpose(0, 2, 1).reshape(L, B, S, F)
    recon = np.concatenate([r["recon"] for r in res.results], axis=1)  # [L, TOK, H]
    recon = recon.astype(np.float32).reshape(L, B, S, H)
    return feats, recon


# revision 8
# speedup vs baseline: 1.0524x; 1.0102x over previous
"""CrossLayerTranscoder Trainium2 kernel.

Math (reference):
  feats = relu(einsum('lbnh,lfh->lbnf', resid, enc_w))            [L,B,S,F]
  recon[t] = sum_{s<=t} feats[s] @ dec_w[s,t].T                   [L,B,S,H]

Sharding: token dim (B*S = 2048) is fully data-parallel through both
stages (recon couples layers/features, never tokens), so each of the 8
cores handles 256 tokens end-to-end with zero communication and zero
redundant compute.  Each core streams the 136 causal (s,t) dec_w pairs
(t-major) from its own HBM copy.

Compute is bf16 (4x faster than fp32 on the PE; accumulation stays f32
in PSUM).  Host pre-transposes operands into PE-friendly layouts:
  residT  [L, 128p, 16a, tok]   h = a*128+p   (encode rhs, moving)
  enc_wT  [L, 128p, 16a, F]     h = a*128+p   (encode lhsT, stationary)
  dec_wT  [136, 4k, 128p, H]    f = k*128+p   (decode rhs, moving)
Encode produces feats in [f, tok] layout so decode can use feats tiles
directly as the stationary operand: recon[tok, h] += feats[f, tok].T @
dec_w[f, h].
"""

import sys

sys.path.insert(0, "/opt/trn_rl_repo")

import os
import numpy as np
import ml_dtypes

L, B, S, H, F = 16, 4, 512, 2048, 512
NCORES = 8
TOK = B * S               # 2048 tokens
TOKC = TOK // NCORES      # 256 tokens per core
HT = H // 128             # 16 h-tiles (encode contraction)
FT = F // 128             # 4 f-tiles (decode contraction)
NPAIRS = L * (L + 1) // 2 # 136 causal (s,t) pairs
HB = H // 512             # 4 h-blocks of 512 (decode moving free dim)
NTT = TOKC // 128         # 2 token tiles of 128 (decode output partitions)

BF16 = ml_dtypes.bfloat16

# pair index p(t,s) = t(t+1)/2 + s, ordered t-major then s ascending
PAIRS = [(s, t) for t in range(L) for s in range(t + 1)]


def _build_nc():
    import concourse.bass as bass
    import concourse.bacc as bacc
    import concourse.mybir as mybir
    from concourse import tile

    f32 = mybir.dt.float32
    bf16 = mybir.dt.bfloat16

    nc = bacc.Bacc()
    residT = nc.declare_dram_parameter("residT", [L, 128, HT, TOKC], bf16, isOutput=False)
    enc_wT = nc.declare_dram_parameter("enc_wT", [L, 128, HT, F], bf16, isOutput=False)
    dec_wT = nc.declare_dram_parameter("dec_wT", [NPAIRS, FT, 128, H], bf16, isOutput=False)
    feats_o = nc.declare_dram_parameter("feats", [L, F, TOKC], bf16, isOutput=True)
    recon_o = nc.declare_dram_parameter("recon", [L, TOKC, H], bf16, isOutput=True)

    with tile.TileContext(nc) as tc:
        with (
            tc.tile_pool(name="rp", bufs=3) as rp,
            tc.tile_pool(name="ep", bufs=2) as ep,
            tc.tile_pool(name="fp", bufs=L * FT) as fp,
            tc.tile_pool(name="dp", bufs=16) as dp,
            tc.tile_pool(name="op", bufs=3) as op,
            tc.tile_pool(name="ps", bufs=8, space=bass.MemorySpace.PSUM) as ps,
        ):
            feats_tiles = {}
            _dma_rr = [0]

            def dma(out_ap, in_ap):
                eng = nc.sync if _dma_rr[0] % 2 == 0 else nc.scalar
                _dma_rr[0] += 1
                eng.dma_start(out_ap, in_ap)

            def encode(s):
                rt = rp.tile([128, HT, TOKC], bf16, tag="rt", name=f"rt{s}")
                dma(rt[:, 0:HT // 2, :], residT[s, :, 0:HT // 2, :])
                dma(rt[:, HT // 2:, :], residT[s, :, HT // 2:, :])
                et = ep.tile([128, HT, F], bf16, tag="et", name=f"et{s}")
                dma(et[:, 0:HT // 2, :], enc_wT[s, :, 0:HT // 2, :])
                dma(et[:, HT // 2:, :], enc_wT[s, :, HT // 2:, :])
                for f in range(FT):
                    acc = ps.tile([128, TOKC], f32, tag="ps", name=f"acc{s}_{f}")
                    for k in range(HT):
                        nc.tensor.matmul(
                            acc[:],
                            et[:, k, f * 128:(f + 1) * 128],
                            rt[:, k, :],
                            start=(k == 0),
                            stop=(k == HT - 1),
                        )
                    ft = fp.tile([128, TOKC], bf16, tag="ft", name=f"ft{s}_{f}")
                    nc.scalar.activation(ft[:], acc[:], mybir.ActivationFunctionType.Relu)
                    dma(feats_o[s, f * 128:(f + 1) * 128, :], ft[:])
                    feats_tiles[(s, f)] = ft

            def wave(t):
                banks = [
                    ps.tile([128, 512], f32, tag="ps", name=f"bank{t}_{i}")
                    for i in range(NTT * HB)
                ]
                for s in range(t + 1):
                    pair = t * (t + 1) // 2 + s
                    for k in range(FT):
                        dw = dp.tile([128, H], bf16, tag="dw", name=f"dw{t}_{s}_{k}")
                        dma(dw[:], dec_wT[pair, k])
                        first = (s == 0 and k == 0)
                        last = (s == t and k == FT - 1)
                        for tt in range(NTT):
                            lhsT = feats_tiles[(s, k)][:, tt * 128:(tt + 1) * 128]
                            for hb in range(HB):
                                nc.tensor.matmul(
                                    banks[tt * HB + hb][:],
                                    lhsT,
                                    dw[:, hb * 512:(hb + 1) * 512],
                                    start=first,
                                    stop=last,
                                )
                for tt in range(NTT):
                    stg = op.tile([128, H], bf16, tag="stg", name=f"stg{t}_{tt}")
                    for hb in range(HB):
                        nc.vector.tensor_copy(
                            stg[:, hb * 512:(hb + 1) * 512], banks[tt * HB + hb][:]
                        )
                    dma(recon_o[t, tt * 128:(tt + 1) * 128, :], stg[:])

            # stagger: encode(t+1) sits between wave(t-1) and wave(t), so
            # PSUM banks of the previous wave drain while encode runs, and
            # decode matmuls keep PE busy while encode inputs stream in.
            encode(0)
            encode(1)
            for t in range(L):
                wave(t)
                if t + 2 < L:
                    encode(t + 2)

    nc.finalize()
    return nc


_NC_CACHE = None


def _get_nc():
    global _NC_CACHE
    if _NC_CACHE is None:
        _NC_CACHE = _build_nc()
    return _NC_CACHE


def _prep_inputs(resid, enc_w, dec_w):
    # residT per core: [L, 128, HT, TOKC], h = a*128 + p
    rs = resid.reshape(L, TOK, HT, 128).transpose(0, 3, 2, 1).astype(BF16)
    resid_shards = [np.ascontiguousarray(rs[..., c * TOKC:(c + 1) * TOKC]) for c in range(NCORES)]
    # enc_wT: [L, 128, HT, F]
    ew = np.ascontiguousarray(enc_w.reshape(L, F, HT, 128).transpose(0, 3, 2, 1)).astype(BF16)
    # dec_wT: [NPAIRS, FT, 128, H], f = k*128 + p
    dw = np.empty((NPAIRS, FT, 128, H), dtype=BF16)
    for i, (s, t) in enumerate(PAIRS):
        dw[i] = dec_w[s, t].T.astype(BF16).reshape(FT, 128, H)
    return resid_shards, ew, dw


def kernel(resid, enc_w, dec_w):
    from concourse.bass_utils import run_bass_kernel_spmd

    resid_shards, ew, dw = _prep_inputs(resid, enc_w, dec_w)
    nc = _get_nc()
    in_maps = [
        {"residT": resid_shards[c], "enc_wT": ew, "dec_wT": dw} for c in range(NCORES)
    ]
    trace = os.environ.get("TRN_KERNEL_PROFILE", "") == "1"
    res = run_bass_kernel_spmd(nc, in_maps, list(range(NCORES)), trace=trace)
    if trace:
        print(f"HW exec time: {res.exec_time_ns} ns")
        kernel.last_exec_time_ns = res.exec_time_ns
        kernel.last_results = res

    feats = np.concatenate([r["feats"] for r in res.results], axis=2)  # [L, F, TOK]
    feats = feats.astype(np.float32).transpose(0, 2, 1).reshape(L, B, S, F)
    recon = np.concatenate([r["recon"] for r in res.results], axis=1)  # [L, TOK, H]
    recon = recon.astype(np.float32).reshape(L, B, S, H)
    return feats, recon
